# revision 1
# baseline (speedup 1.0000x reference)
"""Trainium2 Bass kernel for nn_AdvancedAutoInformerModel.

Math (same folding as the baseline): only the last block (10 tokens) of each
batch feeds the output, so all linear stages fold into small matrices on the
host.  Per-core work (8 cores, 4 batches each): 40 token-columns, D=256.

Structure (vs the 34.5us baseline):
  * Block masking happens INSIDE the QK matmul: per-bank accumulating
    matmuls add A*[same-block] (A = 20/SCALE) to the raw scores and the Exp
    activation gets bias -C, so cross-block exponentials are a factor
    e(-20) down -> no mask tensor_mul anywhere.  Exponentials are bf16
    (raw scores span [-69, +104]; fp16 would overflow).
  * Scores live in 4 psum banks, one per PE row-group (concurrent
    row-tiled matmuls writing the same psum bank+partitions hard-fault);
    ONE strided Exp covers all 8 heads.
  * Softmax denominators are produced FEATURE-major by col-tiled ones-
    matmuls so 1/den fuses into the PSUM->SBUF move of the attention
    output; the AV matmuls run on the unnormalized exponentials (fp16 V x
    bf16 exp -- 16-bit dtype mixing is supported, 32x16 is not).
  * LayerNorm head uses scalar_tensor_tensor accum_out for sum(h2) /
    sum(h2^2) and a fused-STT affine tail.
  * 8 junk warmup matmuls run during the DMA wait so the PE HAM clock
    gate opens (1.2 -> 2.4 GHz) before the real matmuls.
  * q/k projections per layer share one psum tile -> one bias-add each.
"""

import math
import os
import sys

import numpy as np
import ml_dtypes

for _p in ("/opt/trn_rl_repo",):
    if _p not in sys.path:
        sys.path.insert(0, _p)

import concourse.bass as bass
import concourse.bacc as bacc
import concourse.tile as tile
from concourse import mybir
from concourse.bass_utils import run_bass_kernel_spmd

F32 = mybir.dt.float32
F32R = mybir.dt.float32r
FP16 = mybir.dt.float16
BF16 = mybir.dt.bfloat16
NPBF16 = ml_dtypes.bfloat16

B, S, F, D, O, H, BS = 32, 2000, 16, 256, 4, 8, 10
HD = D // H                     # 32
NCORES = 8
BL = B // NCORES                # 4 batches per core
NT = BL * BS                    # 40 token-columns per core
SCALE = 1.0 / math.sqrt(HD)
ABIAS = 20.0 / SCALE            # block bias in raw-score units
MBLK = math.sqrt(ABIAS)         # one-hot scale so (M e)^T (M e) = ABIAS
CEXP = 16.5                     # global shift inside the exp

# xa  per-core f32r [80, 808] = xcolT(40) | aq1(256) | ak1(256) | av1(256)
# cb1 common f32r [128, 884] = bqk1(160) | bv1t(256) | e1k(40) | e1q(320) |
#                  e2q(32) | negC(1) | eps(1) | pad
# cb2 common f32r [128, 2668] = bq2(512) | bk2(512) | bv2(512) | bqk2(88) |
#                  cv2t(256) | wo2t(512) | w2g(8) | bo2t(256) | bgt | gfct | cft
XA_SHAPE = (80, 808)
CB1_SHAPE = (128, 884)
CB2_SHAPE = (128, 2668)

_MODULE_CACHE = {}
LAST_RUN = {}
STAGE = 99       # truncate pipeline for HW bisection (debug only)
WARMUP = 8       # junk matmuls to open the PE clock gate


def _build_module():
    nc = bacc.Bacc()
    xa_p = nc.declare_dram_parameter("xa", list(XA_SHAPE), F32R, isOutput=False)
    cb1_p = nc.declare_dram_parameter("cb1", list(CB1_SHAPE), F32R, isOutput=False)
    cb2_p = nc.declare_dram_parameter("cb2", list(CB2_SHAPE), F32R, isOutput=False)
    out_p = nc.declare_dram_parameter("out", [BL, O], F32, isOutput=True)

    with tile.TileContext(nc) as tc:
        with (
            tc.tile_pool(name="w", bufs=1) as wp,
            tc.tile_pool(name="act", bufs=1) as ap,
            tc.tile_pool(name="ps", bufs=1, space="PSUM") as pp,
        ):
            # ---- parameter DMAs: sync queue carries the layer-1-critical
            # data, gpsimd queue the later-needed blobs ----
            xa = wp.tile(list(XA_SHAPE), F32R, tag="xa", name="xa")
            cb1 = wp.tile(list(CB1_SHAPE), F32R, tag="cb1", name="cb1")
            cb2 = wp.tile(list(CB2_SHAPE), F32R, tag="cb2", name="cb2")
            nc.sync.dma_start(out=xa, in_=xa_p[:])
            nc.sync.dma_start(out=cb1, in_=cb1_p[:])
            nc.gpsimd.dma_start(out=cb2[:, 0:1624], in_=cb2_p[:, 0:1624])
            nc.gpsimd.dma_start(out=cb2[:, 1624:], in_=cb2_p[:, 1624:])

            t = {
                "xcolT": xa[:, 0:40],
                "aq1": xa[:, 40:296], "ak1": xa[:, 296:552],
                "av1": xa[:, 552:808],
                "bqk1": cb1[:, 0:160],
                "bv1t": cb1[0:NT, 160:416],
                # e1k/e1q/e2q replicated at partitions rg*32 (rg=0..3) so
                # each bank's bias matmul runs on that bank's row-group
                "e1k": lambda rg: cb1[rg * 32:rg * 32 + BL, 416:456],
                "e1q": lambda rg: cb1[rg * 32:rg * 32 + BL, 456:776],
                "e2q": lambda rg: cb1[rg * 32:rg * 32 + BL, 776:808],
                "negC": cb1[0:NT, 808:809],
                "eps": cb1[0:BL, 809:810],
                "bq2": cb2[:, 0:512].rearrange("p (c x) -> p c x", c=2),
                "bk2": cb2[:, 512:1024].rearrange("p (c x) -> p c x", c=2),
                "bv2": cb2[:, 1024:1536].rearrange("p (c x) -> p c x", c=2),
                "bqk2": cb2[:, 1536:1624],
                "cv2t": cb2[0:NT, 1624:1880],
                "wo2t": cb2[:, 1880:2392].rearrange("p (c x) -> p c x", c=2),
                "w2g": cb2[:, 2392:2400].rearrange("p (c x) -> p c x", c=2),
                "bo2t": cb2[0:BL, 2400:2656],
                "bgt": cb2[0:BL, 2656:2660],
                "gfct": cb2[0:BL, 2660:2664],
                "cft": cb2[0:BL, 2664:2668],
            }

            def _early(probe):
                nc.gpsimd.dma_start(out=out_p[:], in_=probe)
            if STAGE <= 1:
                _early(cb1[0:BL, 0:O]); return nc

            # bf16 constants: ones [40, 32] for the denominator matmuls and
            # a junk [40, 512] operand for the PE warmup matmuls
            onesc = ap.tile([NT, HD], BF16, tag="onesc")
            nc.gpsimd.memset(onesc, 1.0)
            junk = ap.tile([NT, 512], BF16, tag="junk")
            nc.vector.memset(junk, 1.0)

            # ================= layer 1 =================
            # Scores live in FOUR psum banks, one per PE row-group: bank rg
            # holds heads (rg, rg+4) at cols (0:nq, nq:2nq).
            st1 = pp.tile([NT, 4, 512], F32, tag="st", name="st1",
                          padded_shape=[NT, 4, 512])
            qk1ps = pp.tile([128, 160], F32, tag="qkps", name="qk1ps",
                            padded_shape=[128, 512])
            v1ps = pp.tile([NT, D], F32, tag="vps", name="v1ps",
                           padded_shape=[NT, 512])

            # PE warmup: ~3.5us of junk matmuls while the DMAs land, so the
            # HAM clock gate opens before the real work.  They write into
            # the banks the real layer-1 matmuls use, so same-tile WAW
            # ordering pins them FIRST on the PE.
            for wi in range(WARMUP):
                nc.tensor.matmul(st1[0:HD, wi % 4, :], onesc, junk,
                                 start=True, stop=True, skip_group_check=True)
            nc.tensor.matmul(qk1ps[0:HD, :], onesc, junk[:, 0:160],
                             start=True, stop=True, skip_group_check=True)
            nc.tensor.matmul(v1ps[0:HD, :], onesc, junk[:, 0:256],
                             start=True, stop=True, skip_group_check=True)

            for rg in range(4):
                nc.tensor.matmul(st1[:, rg, 0:2 * NT], t["e1k"](rg),
                                 t["e1q"](rg)[:, 0:2 * NT], start=True,
                                 stop=False, skip_group_check=True,
                                 tile_position=(rg * 32, 0))

            # q1 | k1 feature-major into one PSUM tile [128, 160]
            for mc in range(2):
                nc.tensor.matmul(qk1ps[:, mc * 40:(mc + 1) * 40],
                                 t["aq1"][:, mc * 128:(mc + 1) * 128],
                                 t["xcolT"], start=True, stop=True)
            for mc in range(2):
                nc.tensor.matmul(qk1ps[:, 80 + mc * 40:120 + mc * 40],
                                 t["ak1"][:, mc * 128:(mc + 1) * 128],
                                 t["xcolT"], start=True, stop=True)
            # v1 token-major [40, 256]
            nc.tensor.matmul(v1ps, t["xcolT"], t["av1"], start=True, stop=True)

            qk1 = ap.tile([128, 160], F32R, tag="qk1")
            nc.vector.tensor_add(qk1, qk1ps, t["bqk1"])
            v1 = ap.tile([NT, D], FP16, tag="v1")
            nc.vector.tensor_add(v1, v1ps, t["bv1t"])

            if STAGE <= 2:
                _early(qk1[0:BL, 0:O]); return nc

            def attention(qk, v, st, nq, tag):
                """qk: [128, 2*nq | 2*NT] feature-major (q cols first),
                v: [NT, 256] fp16 token-major, st: [NT, 4, 512] scores psum
                (bank rg = heads rg, rg+4; block-bias already accumulated).
                Returns (o_chunk0, o_chunk1) [128, nq] f32r feature-major."""
                # 8 QK matmuls accumulate into st (row-tiled, f32r ok);
                # head h -> bank h%4, col (h//4)*nq
                for h in range(H):
                    c, pb = h // 4, (h % 4) * HD
                    nc.tensor.matmul(
                        st[:, h % 4, c * nq:(c + 1) * nq],
                        qk[pb:pb + HD, 2 * nq + c * NT:2 * nq + (c + 1) * NT],
                        qk[pb:pb + HD, c * nq:(c + 1) * nq],
                        start=False, stop=True,
                        tile_position=(pb, 0), skip_group_check=True,
                    )
                # single Exp for all heads: exp(SCALE*s - C), bf16 out
                et = ap.tile([NT, 4, 2 * nq], BF16, tag=tag + "_et")
                nc.scalar.activation(et, st[:, :, 0:2 * nq],
                                     mybir.ActivationFunctionType.Exp,
                                     scale=SCALE, bias=t["negC"])
                # denominators + weighted V, feature-major; head h = c*4+hh
                # lives in et[:, hh, c*nq:(c+1)*nq].  cs/av are single-bank
                # tiles shared by both chunks (col-groups repeat -> serial).
                cs = pp.tile([128, 2, nq], F32, tag="cs", name=tag + "cs",
                             padded_shape=[128, 2, 256])
                av = pp.tile([128, 2, nq], F32, tag="av", name=tag + "av",
                             padded_shape=[128, 2, 256])
                for c in range(2):
                    for hh in range(4):
                        nc.tensor.matmul(
                            cs[hh * HD:(hh + 1) * HD, c, :], onesc,
                            et[:, hh, c * nq:(c + 1) * nq],
                            start=True, stop=True, tile_position=(0, hh * HD),
                        )
                    for hh in range(4):
                        h = c * 4 + hh
                        nc.tensor.matmul(
                            av[hh * HD:(hh + 1) * HD, c, :],
                            v[0:NT, h * HD:(h + 1) * HD],
                            et[:, hh, c * nq:(c + 1) * nq],
                            start=True, stop=True, tile_position=(0, hh * HD),
                        )
                outs = []
                for c in range(2):
                    rb = ap.tile([128, nq], F32, tag=tag + f"_rb{c}")
                    nc.vector.reciprocal_approx_fast(out=rb, in_=cs[:, c, :])
                    o = ap.tile([128, nq], F32R, tag=tag + f"_o{c}")
                    nc.vector.tensor_mul(o, av[:, c, :], rb)
                    outs.append(o)
                return outs

            o1 = attention(qk1, v1, st1, NT, "l1")
            if STAGE <= 3:
                _early(o1[0][0:BL, 0:O]); return nc

            # ================= layer 2 =================
            # q2 (last token of each batch) + k2 share one PSUM tile.
            # NB: start=True resets has_written for the WHOLE psum bank, so
            # each col-region's accumulation group must complete before the
            # next region's start (mc outer, kc inner).
            qk2ps = pp.tile([128, 88], F32, tag="qkps", name="qk2ps",
                            padded_shape=[128, 512])
            v2ps = pp.tile([NT, D], F32, tag="vps", name="v2ps",
                           padded_shape=[NT, 512])
            ols = [o1[kc].rearrange("p (b t) -> p b t", t=BS)[:, :, BS - 1]
                   for kc in range(2)]
            for mc in range(2):
                for kc in range(2):
                    nc.tensor.matmul(qk2ps[:, mc * 4:(mc + 1) * 4],
                                     t["bq2"][:, kc, mc * 128:(mc + 1) * 128],
                                     ols[kc], start=(kc == 0), stop=(kc == 1),
                                     skip_group_check=True)
            for mc in range(2):
                for kc in range(2):
                    nc.tensor.matmul(qk2ps[:, 8 + mc * 40:48 + mc * 40],
                                     t["bk2"][:, kc, mc * 128:(mc + 1) * 128],
                                     o1[kc], start=(kc == 0), stop=(kc == 1),
                                     skip_group_check=True)
            for kc in range(2):
                nc.tensor.matmul(v2ps, o1[kc], t["bv2"][:, kc, :],
                                 start=(kc == 0), stop=(kc == 1))

            qk2 = ap.tile([128, 88], F32R, tag="qk2")
            nc.vector.tensor_add(qk2, qk2ps, t["bqk2"])
            v2 = ap.tile([NT, D], FP16, tag="v2")
            nc.vector.tensor_add(v2, v2ps, t["cv2t"])

            st2 = pp.tile([NT, 4, 512], F32, tag="st", name="st2",
                          padded_shape=[NT, 4, 512])
            for rg in range(4):
                nc.tensor.matmul(st2[:, rg, 0:2 * BL], t["e1k"](rg),
                                 t["e2q"](rg)[:, 0:2 * BL], start=True,
                                 stop=False, skip_group_check=True,
                                 tile_position=(rg * 32, 0))
            o2 = attention(qk2, v2, st2, BL, "l2")
            if STAGE <= 5:
                _early(o2[0][0:BL, 0:O]); return nc

            # ================= head =================
            # h2 [4, 256] token-major + t2 [4, 4] in one PSUM tile
            hd = pp.tile([BL, D + O], F32, tag="qkps", name="headps",
                         padded_shape=[BL, 512])
            for kc in range(2):
                nc.tensor.matmul(hd[:, 0:D], o2[kc], t["wo2t"][:, kc, :],
                                 start=(kc == 0), stop=(kc == 1))
            for kc in range(2):
                nc.tensor.matmul(hd[:, D:D + O], o2[kc], t["w2g"][:, kc, :],
                                 start=(kc == 0), stop=(kc == 1))

            M = mybir.AluOpType.mult
            ADD = mybir.AluOpType.add
            SUB = mybir.AluOpType.subtract
            h2 = ap.tile([BL, D], F32, tag="h2")
            su = ap.tile([BL, 1], F32, tag="su")
            nc.vector.scalar_tensor_tensor(out=h2, in0=hd[:, 0:D], scalar=1.0,
                                           in1=t["bo2t"], op0=M, op1=ADD,
                                           accum_out=su)
            sq = ap.tile([BL, D], F32, tag="sq")
            sqs = ap.tile([BL, 1], F32, tag="sqs")
            nc.vector.scalar_tensor_tensor(out=sq, in0=h2, scalar=1.0, in1=h2,
                                           op0=M, op1=M, accum_out=sqs)
            # var = sqs/256 - (su/256)^2 ; rstd = 1/sqrt(var + eps)
            musq = ap.tile([BL, 1], F32, tag="musq")
            nc.vector.tensor_scalar(out=musq, in0=su, scalar1=su,
                                    scalar2=1.0 / (D * D), op0=M, op1=M)
            var = ap.tile([BL, 1], F32, tag="var")
            nc.vector.scalar_tensor_tensor(out=var, in0=sqs, scalar=1.0 / D,
                                           in1=musq, op0=M, op1=SUB)
            std = ap.tile([BL, 1], F32, tag="std")
            nc.scalar.activation(std, var, mybir.ActivationFunctionType.Sqrt,
                                 bias=t["eps"])
            rstd = ap.tile([BL, 1], F32, tag="rstd")
            nc.vector.reciprocal_approx_fast(out=rstd, in_=std)
            u = ap.tile([BL, 1], F32, tag="u")
            nc.vector.tensor_scalar(out=u, in0=rstd, scalar1=su,
                                    scalar2=1.0 / D, op0=M, op1=M)
            # r2 = rstd*t2 + (rstd*bgt - (u*gfct - cft))
            bm = ap.tile([BL, O], F32, tag="bm")
            nc.vector.scalar_tensor_tensor(out=bm, in0=t["gfct"], scalar=u,
                                           in1=t["cft"], op0=M, op1=SUB)
            q1t = ap.tile([BL, O], F32, tag="q1t")
            nc.vector.scalar_tensor_tensor(out=q1t, in0=t["bgt"], scalar=rstd,
                                           in1=bm, op0=M, op1=SUB)
            r2 = ap.tile([BL, O], F32, tag="r2")
            nc.vector.scalar_tensor_tensor(out=r2, in0=hd[:, D:D + O],
                                           scalar=rstd, in1=q1t,
                                           op0=M, op1=ADD)
            nc.sync.dma_start(out=out_p[:], in_=r2)

    return nc


def _host_fold(inputs):
    """Fold all linear stages; returns (cb1, cb2, xa per-core list)."""
    g = {k: np.asarray(v, np.float32) for k, v in inputs.items()}
    x = g["x"]

    Wm = np.zeros((5, F, D), np.float32)
    w1, w3, w5 = g["conv_w1"], g["conv_w3"], g["conv_w5"]
    Wm[0] = w5[0]
    Wm[1] = w3[0] + w5[1]
    Wm[2] = w1[0] + w3[1] + w5[2]
    Wm[3] = w3[2] + w5[3]
    Wm[4] = w5[4]
    Wm = Wm.reshape(80, D)
    bm = g["conv_b1"] + g["conv_b3"] + g["conv_b5"]

    toks = np.arange(S - BS, S)
    pos = toks.astype(np.float32)[:, None]
    div = np.exp(np.arange(0, D, 2, dtype=np.float32) * (-math.log(10000.0) / D))
    pe = np.zeros((BS, D), np.float32)
    pe[:, 0::2] = np.sin(pos * div)
    pe[:, 1::2] = np.cos(pos * div)

    Mts = np.eye(D, dtype=np.float32) + g["trend_w"].sum(0) + g["seas_w"].sum(0)
    bts = g["trend_b"].sum(0) + g["seas_b"].sum(0)

    WmM = Wm @ Mts                       # [80, 256]
    c0 = (bm[None] + pe) @ Mts + bts     # [10, 256]

    wqkv, bqkv = g["attn_wqkv"], g["attn_bqkv"]
    wo, bo = g["attn_wo"], g["attn_bo"]
    Wq1, Wk1, Wv1 = wqkv[0][:D], wqkv[0][D:2 * D], wqkv[0][2 * D:]
    bq1, bk1, bv1 = bqkv[0][:D], bqkv[0][D:2 * D], bqkv[0][2 * D:]
    Wq2, Wk2, Wv2 = wqkv[1][:D], wqkv[1][D:2 * D], wqkv[1][2 * D:]
    bq2, bk2, bv2 = bqkv[1][:D], bqkv[1][D:2 * D], bqkv[1][2 * D:]
    Wo1, bo1, Wo2, bo2 = wo[0], bo[0], wo[1], bo[1]

    def fm_chunks(w):          # [256, X] -> [128, 2, X] (K/feature chunks)
        return np.ascontiguousarray(
            w.reshape(2, 128, w.shape[1]).transpose(1, 0, 2))

    def biastab_fm80(tab):     # [10, 256] -> [128, 80] (chunk-major, batch-tiled)
        a = tab.T.reshape(2, 128, BS).transpose(1, 0, 2)   # [128, 2, 10]
        return np.ascontiguousarray(np.broadcast_to(
            a[:, :, None, :], (128, 2, BL, BS)).reshape(128, 80))

    def bias_fm88(vq, vk):     # feature-const biases -> [128, 88]
        out = np.zeros((128, 88), np.float32)
        q = vq.reshape(2, 128).T                            # [128, 2]
        k = vk.reshape(2, 128).T
        out[:, 0:4] = q[:, 0:1]
        out[:, 4:8] = q[:, 1:2]
        out[:, 8:48] = k[:, 0:1]
        out[:, 48:88] = k[:, 1:2]
        return out

    blk = (np.arange(NT) // BS)                             # [40] batch of token
    e1k = MBLK * (blk[None, :] == np.arange(BL)[:, None])   # [4, 40]
    e1q = np.tile(e1k, (1, H))                              # [4, 320]
    e2q = np.tile(MBLK * np.eye(BL, dtype=np.float32), (1, H))  # [4, 32]

    cb1 = np.zeros(CB1_SHAPE, np.float32)
    cb1[:, 0:80] = biastab_fm80(c0 @ Wq1.T + bq1)
    cb1[:, 80:160] = biastab_fm80(c0 @ Wk1.T + bk1)
    cb1[0:NT, 160:416] = np.tile(c0 @ Wv1.T + bv1, (BL, 1))
    for rg in range(4):
        cb1[rg * 32:rg * 32 + BL, 416:456] = e1k
        cb1[rg * 32:rg * 32 + BL, 456:776] = e1q
        cb1[rg * 32:rg * 32 + BL, 776:808] = e2q
    cb1[:, 808] = -CEXP
    cb1[0:BL, 809] = 1e-5

    G = g["ln_g"][:, None] * g["fc_w"]
    cb2 = np.zeros(CB2_SHAPE, np.float32)
    cb2[:, 0:512] = fm_chunks(Wo1.T @ Wq2.T).reshape(128, 512)
    cb2[:, 512:1024] = fm_chunks(Wo1.T @ Wk2.T).reshape(128, 512)
    cb2[:, 1024:1536] = fm_chunks(Wo1.T @ Wv2.T).reshape(128, 512)
    cb2[:, 1536:1624] = bias_fm88(bo1 @ Wq2.T + bq2, bo1 @ Wk2.T + bk2)
    cb2[0:NT, 1624:1880] = np.tile(bo1 @ Wv2.T + bv2, (NT, 1))
    cb2[:, 1880:2392] = fm_chunks(Wo2.T).reshape(128, 512)
    cb2[:, 2392:2400] = fm_chunks(Wo2.T @ G).reshape(128, 8)
    cb2[0:BL, 2400:2656] = np.tile(bo2, (BL, 1))
    cb2[0:BL, 2656:2660] = np.tile(bo2 @ G, (BL, 1))
    cb2[0:BL, 2660:2664] = np.tile(g["ln_g"] @ g["fc_w"], (BL, 1))
    cb2[0:BL, 2664:2668] = np.tile(g["ln_b"] @ g["fc_w"] + g["fc_b"], (BL, 1))

    com_q1 = WmM @ Wq1.T
    com_k1 = WmM @ Wk1.T
    com_v1 = WmM @ Wv1.T

    # im2col of the live tokens: windows x[1988+t : 1993+t], t=0..9
    xp = np.zeros((B, S + 2, F), np.float32)
    xp[:, :S] = x
    xcol = np.stack([xp[:, S - BS - 2 + tt:S - BS + 3 + tt, :].reshape(B, 80)
                     for tt in range(BS)], 1)            # [B, 10, 80]
    xas = []
    for c in range(NCORES):
        xa = np.zeros(XA_SHAPE, np.float32)
        xa[:, 0:40] = xcol[c * BL:(c + 1) * BL].transpose(2, 0, 1).reshape(80, NT)
        xa[:, 40:296] = com_q1
        xa[:, 296:552] = com_k1
        xa[:, 552:808] = com_v1
        xas.append(xa)
    return cb1, cb2, xas


def kernel(**inputs):
    cb1, cb2, xas = _host_fold(inputs)
    if "nc" not in _MODULE_CACHE:
        nc = _build_module()
        if not nc.is_finalized():
            nc.finalize()
        _MODULE_CACHE["nc"] = nc
    nc = _MODULE_CACHE["nc"]
    in_maps = [{"cb1": cb1, "cb2": cb2, "xa": xas[c]} for c in range(NCORES)]
    trace = bool(os.environ.get("KERNEL_TRACE"))
    res = run_bass_kernel_spmd(nc, in_maps, core_ids=list(range(NCORES)),
                               trace=trace)
    LAST_RUN["result"] = res
    out = np.concatenate([res.results[c]["out"] for c in range(NCORES)], 0)
    return out.astype(np.float32)



# revision 8
# speedup vs baseline: 1.1992x; 1.1992x over previous
"""Trainium2 Bass kernel for nn_AdvancedAutoInformerModel.

Math (same folding as before): only the last block (10 tokens) of each batch
feeds the output, so all linear stages fold into small matrices on the host.
Per-core work (8 cores, 4 batches each): 40 token-columns, D=256.

v2 structure (vs the 31.5us baseline):
  * Everything the PE touches is fp16/bf16: fp16 streams 1 col/cycle vs 4
    for f32r on <256-col matmuls, LDWEIGHTS bytes halve, and the input DMA
    drops from 2.08MB to ~0.87MB per core.
  * DMA priority: xa (im2col + L1 weights) rides the sync ring alone while
    cb1 (bias-matmul e-block, 23KB) leads the gpsimd ring ahead of cb2
    (L2/head weights, 683KB) -- L1 compute starts ~6us earlier and cb2
    streams in during L1.
  * L1 q/k/v biases are folded into the projection matmuls as 10 extra
    im2col rows (onehot(t%10) x bias-table) -- no bias adds, no bias DMA.
  * The e-block holds only the 88 live columns (e1k|e1k, e2q2) instead of
    the 392-column tiling.
  * Sqrt's activation table is preloaded via a dummy activation during the
    DMA window so the LayerNorm tail never waits 1.3us for ACT_TABLE_LOAD.
  * PSUM->SBUF moves and the o=av*rb muls are split across gpsimd/vector;
    exp and sqrt live on scalar.
  * Head biases (bo2t|bgt) enter PSUM via a rank-1 ones matmul that runs as
    the accumulation-group opener, killing two tail STTs.
  * Block masking, 4-bank score tiles, feature-major denominators, the
    fused-STT LayerNorm tail, and the PE warmup are as before.
"""

import math
import os
import sys

import numpy as np
import ml_dtypes

for _p in ("/opt/trn_rl_repo",):
    if _p not in sys.path:
        sys.path.insert(0, _p)

import concourse.bass as bass
import concourse.bacc as bacc
import concourse.tile as tile
from concourse import mybir
from concourse.bass_utils import run_bass_kernel_spmd

F32 = mybir.dt.float32
FP16 = mybir.dt.float16
BF16 = mybir.dt.bfloat16

B, S, F, D, O, H, BS = 32, 2000, 16, 256, 4, 8, 10
HD = D // H                     # 32
NCORES = 8
BL = B // NCORES                # 4 batches per core
NT = BL * BS                    # 40 token-columns per core
KI = 80 + BS                    # im2col rows + 10 bias-onehot rows
SCALE = 1.0 / math.sqrt(HD)
ABIAS = 20.0 / SCALE            # block bias in raw-score units
MBLK = math.sqrt(ABIAS)         # one-hot scale so (M e)^T (M e) = ABIAS
CEXP = 16.5                     # global shift inside the exp

# xa  per-core fp16 [90, 808] = xcolT(40) | aq1(256) | ak1(256) | av1(256)
#     rows 80:90 carry onehot(t%10) (xcolT) / folded bias tables (aq1..av1)
# cb1 common fp16 [128, 88] = e1k(40) | e1k(40) | e2q2(8), replicated at
#     partitions rg*32 (rg=0..3) so each bank's bias matmul reads its rows
# cb2 common fp16 [128, 2796] = bq2(512) | bk2(512) | bv2(512) | wo2t(512) |
#     w2g(8) | cv2row(256) | hb(260) | qkbias4(128) | qkmask4(88) |
#     gfct(4) | cft(4)   (1/4-row blocks live at partition base 0)
XA_SHAPE = (KI, 808)
CB1_SHAPE = (128, 88)
CB2_SHAPE = (128, 2796)

_MODULE_CACHE = {}
LAST_RUN = {}
STAGE = 99       # truncate pipeline for HW bisection (debug only)
WARMUP = 6       # junk matmuls to open the PE clock gate
WCOLS = 256      # columns per junk matmul


def _build_module():
    nc = bacc.Bacc()
    xa_p = nc.declare_dram_parameter("xa", list(XA_SHAPE), FP16, isOutput=False)
    cb1_p = nc.declare_dram_parameter("cb1", list(CB1_SHAPE), FP16, isOutput=False)
    cb2_p = nc.declare_dram_parameter("cb2", list(CB2_SHAPE), FP16, isOutput=False)
    out_p = nc.declare_dram_parameter("out", [BL, O], F32, isOutput=True)

    with tile.TileContext(nc) as tc:
        with (
            tc.tile_pool(name="w", bufs=1) as wp,
            tc.tile_pool(name="act", bufs=1) as ap,
            tc.tile_pool(name="ps", bufs=1, space="PSUM") as pp,
        ):
            # ---- parameter DMAs: sync ring carries xa alone; gpsimd ring
            # carries the tiny e-block first, then the L2/head blob ----
            xa = wp.tile(list(XA_SHAPE), FP16, tag="xa", name="xa")
            cb1 = wp.tile(list(CB1_SHAPE), FP16, tag="cb1", name="cb1")
            cb2 = wp.tile(list(CB2_SHAPE), FP16, tag="cb2", name="cb2")
            nc.sync.dma_start(out=xa, in_=xa_p[:])
            nc.gpsimd.dma_start(out=cb1, in_=cb1_p[:])
            nc.gpsimd.dma_start(out=cb2, in_=cb2_p[:])

            t = {
                "xcolT": xa[:, 0:40],
                "aq1": xa[:, 40:296], "ak1": xa[:, 296:552],
                "av1": xa[:, 552:808],
                "e1k": lambda rg: cb1[rg * 32:rg * 32 + BL, 0:40],
                "e1q": lambda rg: cb1[rg * 32:rg * 32 + BL, 0:80],
                "e2q": lambda rg: cb1[rg * 32:rg * 32 + BL, 80:88],
                "bq2": cb2[:, 0:512].rearrange("p (c x) -> p c x", c=2),
                "bk2": cb2[:, 512:1024].rearrange("p (c x) -> p c x", c=2),
                "bv2": cb2[:, 1024:1536].rearrange("p (c x) -> p c x", c=2),
                "wo2t": cb2[:, 1536:2048].rearrange("p (c x) -> p c x", c=2),
                "w2g": cb2[:, 2048:2056].rearrange("p (c x) -> p c x", c=2),
                "cv2row": cb2[0:1, 2056:2312],
                "hb": cb2[0:1, 2312:2572],
                "qkbias4": cb2[0:4, 2572:2700],
                "qkmask4": cb2[0:4, 2700:2788],
                "gfct": cb2[0:BL, 2788:2792],
                "cft": cb2[0:BL, 2792:2796],
            }

            def _early(probe):
                nc.gpsimd.dma_start(out=out_p[:], in_=probe)
            if STAGE <= 1:
                _early(cb1[0:BL, 0:O]); return nc

            # junk [40, 512] bf16 = warmup rhs; ones views serve the
            # denominator matmuls and the head's rank-1 bias matmul
            junk = ap.tile([NT, 512], BF16, tag="junk")
            nc.vector.memset(junk, 1.0)
            onesc = junk[:, 0:HD]

            # activation bias tiles (const pool only carries 0/1)
            negc = ap.tile([NT, 1], F32, tag="negc")
            nc.gpsimd.memset(negc, -CEXP)
            eps = ap.tile([BL, 1], F32, tag="eps")
            nc.gpsimd.memset(eps, 1e-5)

            # Preload the Sqrt activation table during the DMA window; Exp's
            # table load then also lands before the first real exp.
            scr = ap.tile([1, 1], F32, tag="scr")
            nc.scalar.activation(scr, junk[0:1, 0:1],
                                 mybir.ActivationFunctionType.Sqrt)

            # ================= layer 1 =================
            # Scores live in FOUR psum banks, one per PE row-group: bank rg
            # holds heads (rg, rg+4) at cols (0:nq, nq:2nq).
            st1 = pp.tile([NT, 4, 512], F32, tag="st", name="st1",
                          padded_shape=[NT, 4, 512])
            qk1ps = pp.tile([128, 160], F32, tag="qkps", name="qk1ps",
                            padded_shape=[128, 512])
            v1ps = pp.tile([NT, D], F32, tag="vps", name="v1ps",
                           padded_shape=[NT, 512])

            # PE warmup while the DMAs land so the HAM clock gate opens.
            # They write the banks the real layer-1 matmuls use, so
            # same-tile WAW ordering pins them FIRST on the PE.
            for wi in range(WARMUP - 2):
                nc.tensor.matmul(st1[0:HD, wi % 4, 0:WCOLS], onesc,
                                 junk[:, 0:WCOLS], start=True, stop=True,
                                 skip_group_check=True)
            nc.tensor.matmul(qk1ps[0:HD, :], onesc, junk[:, 0:160],
                             start=True, stop=True, skip_group_check=True)
            nc.tensor.matmul(v1ps[0:HD, :], onesc, junk[:, 0:256],
                             start=True, stop=True, skip_group_check=True)

            for rg in range(4):
                nc.tensor.matmul(st1[:, rg, 0:2 * NT], t["e1k"](rg),
                                 t["e1q"](rg), start=True,
                                 stop=False, skip_group_check=True,
                                 tile_position=(rg * 32, 0))

            # q1 | k1 feature-major into one PSUM tile [128, 160]; biases
            # ride the 10 extra im2col rows
            for mc in range(2):
                nc.tensor.matmul(qk1ps[:, mc * 40:(mc + 1) * 40],
                                 t["aq1"][:, mc * 128:(mc + 1) * 128],
                                 t["xcolT"], start=True, stop=True)
            for mc in range(2):
                nc.tensor.matmul(qk1ps[:, 80 + mc * 40:120 + mc * 40],
                                 t["ak1"][:, mc * 128:(mc + 1) * 128],
                                 t["xcolT"], start=True, stop=True)
            # v1 token-major [40, 256]
            nc.tensor.matmul(v1ps, t["xcolT"], t["av1"], start=True, stop=True)

            qk1 = ap.tile([128, 160], FP16, tag="qk1")
            nc.scalar.copy(out=qk1, in_=qk1ps)
            v1 = ap.tile([NT, D], FP16, tag="v1")
            nc.vector.tensor_copy(out=v1, in_=v1ps)

            if STAGE <= 2:
                _early(qk1[0:BL, 0:O]); return nc

            def attention(qk, v, st, nq, tag):
                """qk: [128, 2*nq | 2*NT] feature-major (q cols first),
                v: [NT, 256] fp16 token-major, st: [NT, 4, 512] scores psum
                (bank rg = heads rg, rg+4; block-bias already accumulated).
                Returns (o_chunk0, o_chunk1) [128, nq] fp16 feature-major."""
                # 8 QK matmuls accumulate into st (row-tiled);
                # head h -> bank h%4, col (h//4)*nq
                for h in range(H):
                    c, pb = h // 4, (h % 4) * HD
                    nc.tensor.matmul(
                        st[:, h % 4, c * nq:(c + 1) * nq],
                        qk[pb:pb + HD, 2 * nq + c * NT:2 * nq + (c + 1) * NT],
                        qk[pb:pb + HD, c * nq:(c + 1) * nq],
                        start=False, stop=True,
                        tile_position=(pb, 0), skip_group_check=True,
                    )
                # single Exp for all heads: exp(SCALE*s - C), bf16 out
                et = ap.tile([NT, 4, 2 * nq], BF16, tag=tag + "_et")
                nc.scalar.activation(et, st[:, :, 0:2 * nq],
                                     mybir.ActivationFunctionType.Exp,
                                     scale=SCALE, bias=negc)
                # denominators + weighted V, feature-major; head h = c*4+hh
                # lives in et[:, hh, c*nq:(c+1)*nq].  cs/av are single-bank
                # tiles shared by both chunks (col-groups repeat -> serial).
                cs = pp.tile([128, 2, nq], F32, tag="cs", name=tag + "cs",
                             padded_shape=[128, 2, 256])
                av = pp.tile([128, 2, nq], F32, tag="av", name=tag + "av",
                             padded_shape=[128, 2, 256])
                for c in range(2):
                    for hh in range(4):
                        nc.tensor.matmul(
                            cs[hh * HD:(hh + 1) * HD, c, :], onesc,
                            et[:, hh, c * nq:(c + 1) * nq],
                            start=True, stop=True, tile_position=(0, hh * HD),
                        )
                    for hh in range(4):
                        h = c * 4 + hh
                        nc.tensor.matmul(
                            av[hh * HD:(hh + 1) * HD, c, :],
                            v[0:NT, h * HD:(h + 1) * HD],
                            et[:, hh, c * nq:(c + 1) * nq],
                            start=True, stop=True, tile_position=(0, hh * HD),
                        )
                # one reciprocal + one mul cover both chunks (DVE time is
                # overhead-dominated at this size)
                rb = ap.tile([128, 2, nq], F32, tag=tag + "_rb")
                nc.vector.reciprocal_approx_fast(out=rb, in_=cs[:, :, :])
                ob = ap.tile([128, 2, nq], FP16, tag=tag + "_o")
                nc.vector.tensor_mul(ob, av[:, :, :], rb)
                return [ob[:, 0, :], ob[:, 1, :]]

            o1 = attention(qk1, v1, st1, NT, "l1")
            if STAGE <= 3:
                _early(o1[0][0:BL, 0:O]); return nc

            # ================= layer 2 =================
            # q2 (last token of each batch) + k2 share one PSUM tile.
            # NB: start=True resets has_written for the WHOLE psum bank, so
            # each col-region's accumulation group must complete before the
            # next region's start (mc outer, kc inner).
            qk2ps = pp.tile([128, 88], F32, tag="qkps", name="qk2ps",
                            padded_shape=[128, 512])
            v2ps = pp.tile([NT, D], F32, tag="vps", name="v2ps",
                           padded_shape=[NT, 512])
            ols = [o1[kc].rearrange("p (b t) -> p b t", t=BS)[:, :, BS - 1]
                   for kc in range(2)]
            # rank-4 bias opener: qkbias4.T @ qkmask4 paints the whole
            # [128, 88] bias table into PSUM; projections then accumulate
            nc.tensor.matmul(qk2ps[:, 0:88], t["qkbias4"], t["qkmask4"],
                             start=True, stop=False, skip_group_check=True)
            for mc in range(2):
                for kc in range(2):
                    nc.tensor.matmul(qk2ps[:, mc * 4:(mc + 1) * 4],
                                     t["bq2"][:, kc, mc * 128:(mc + 1) * 128],
                                     ols[kc], start=False, stop=False,
                                     skip_group_check=True)
            for mc in range(2):
                for kc in range(2):
                    nc.tensor.matmul(qk2ps[:, 8 + mc * 40:48 + mc * 40],
                                     t["bk2"][:, kc, mc * 128:(mc + 1) * 128],
                                     o1[kc], start=False,
                                     stop=(mc == 1 and kc == 1),
                                     skip_group_check=True)
            nc.tensor.matmul(v2ps, junk[0:1, 0:NT], t["cv2row"],
                             start=True, stop=False, skip_group_check=True)
            for kc in range(2):
                nc.tensor.matmul(v2ps, o1[kc], t["bv2"][:, kc, :],
                                 start=False, stop=(kc == 1),
                                 skip_group_check=True)

            qk2 = ap.tile([128, 88], FP16, tag="qk2")
            nc.scalar.copy(out=qk2, in_=qk2ps)
            v2 = ap.tile([NT, D], FP16, tag="v2")
            nc.vector.tensor_copy(out=v2, in_=v2ps)

            st2 = pp.tile([NT, 4, 512], F32, tag="st", name="st2",
                          padded_shape=[NT, 4, 512])
            for rg in range(4):
                nc.tensor.matmul(st2[:, rg, 0:2 * BL], t["e1k"](rg),
                                 t["e2q"](rg), start=True,
                                 stop=False, skip_group_check=True,
                                 tile_position=(rg * 32, 0))
            o2 = attention(qk2, v2, st2, BL, "l2")
            if STAGE <= 5:
                _early(o2[0][0:BL, 0:O]); return nc

            # ================= head =================
            # h2 [4, 256] token-major + t2 [4, 4] in one PSUM tile; the
            # rank-1 ones x (bo2t|bgt) matmul opens the accumulation group
            # so biases are already inside PSUM.
            hd = pp.tile([BL, D + O], F32, tag="qkps", name="headps",
                         padded_shape=[BL, 512])
            nc.tensor.matmul(hd[:, 0:D + O], junk[0:1, 0:BL], t["hb"],
                             start=True, stop=False, skip_group_check=True)
            for kc in range(2):
                nc.tensor.matmul(hd[:, 0:D], o2[kc], t["wo2t"][:, kc, :],
                                 start=False, stop=False,
                                 skip_group_check=True)
            for kc in range(2):
                nc.tensor.matmul(hd[:, D:D + O], o2[kc], t["w2g"][:, kc, :],
                                 start=False, stop=(kc == 1),
                                 skip_group_check=True)

            M = mybir.AluOpType.mult
            SUB = mybir.AluOpType.subtract
            h2 = ap.tile([BL, D], F32, tag="h2")
            su = ap.tile([BL, 1], F32, tag="su")
            nc.vector.tensor_scalar(out=h2, in0=hd[:, 0:D], scalar1=1.0,
                                    scalar2=0.0, op0=M,
                                    op1=mybir.AluOpType.add, accum_out=su)
            sq = ap.tile([BL, D], F32, tag="sq")
            sqs = ap.tile([BL, 1], F32, tag="sqs")
            nc.vector.scalar_tensor_tensor(out=sq, in0=h2, scalar=1.0, in1=h2,
                                           op0=M, op1=M, accum_out=sqs)
            # var = sqs/256 - (su/256)^2 ; rstd = 1/sqrt(var + eps)
            musq = ap.tile([BL, 1], F32, tag="musq")
            nc.vector.tensor_scalar(out=musq, in0=su, scalar1=su,
                                    scalar2=1.0 / (D * D), op0=M, op1=M)
            var = ap.tile([BL, 1], F32, tag="var")
            nc.vector.scalar_tensor_tensor(out=var, in0=sqs, scalar=1.0 / D,
                                           in1=musq, op0=M, op1=SUB)
            std = ap.tile([BL, 1], F32, tag="std")
            nc.scalar.activation(std, var, mybir.ActivationFunctionType.Sqrt,
                                 bias=eps)
            rstd = ap.tile([BL, 1], F32, tag="rstd")
            nc.vector.reciprocal_approx_fast(out=rstd, in_=std)
            u = ap.tile([BL, 1], F32, tag="u")
            nc.vector.tensor_scalar(out=u, in0=rstd, scalar1=su,
                                    scalar2=1.0 / D, op0=M, op1=M)
            # r2 = rstd*t2' - (u*gfct - cft)   (t2' = o2@W2g + bgt)
            bm = ap.tile([BL, O], F32, tag="bm")
            nc.vector.scalar_tensor_tensor(out=bm, in0=t["gfct"], scalar=u,
                                           in1=t["cft"], op0=M, op1=SUB)
            r2 = ap.tile([BL, O], F32, tag="r2")
            nc.vector.scalar_tensor_tensor(out=r2, in0=hd[:, D:D + O],
                                           scalar=rstd, in1=bm,
                                           op0=M, op1=SUB)
            nc.sync.dma_start(out=out_p[:], in_=r2)

    return nc


def _host_fold(inputs):
    """Fold all linear stages; returns (cb1, cb2, xa per-core list), fp16."""
    g = {k: np.asarray(v, np.float32) for k, v in inputs.items()}
    x = g["x"]

    Wm = np.zeros((5, F, D), np.float32)
    w1, w3, w5 = g["conv_w1"], g["conv_w3"], g["conv_w5"]
    Wm[0] = w5[0]
    Wm[1] = w3[0] + w5[1]
    Wm[2] = w1[0] + w3[1] + w5[2]
    Wm[3] = w3[2] + w5[3]
    Wm[4] = w5[4]
    Wm = Wm.reshape(80, D)
    bm = g["conv_b1"] + g["conv_b3"] + g["conv_b5"]

    toks = np.arange(S - BS, S)
    pos = toks.astype(np.float32)[:, None]
    div = np.exp(np.arange(0, D, 2, dtype=np.float32) * (-math.log(10000.0) / D))
    pe = np.zeros((BS, D), np.float32)
    pe[:, 0::2] = np.sin(pos * div)
    pe[:, 1::2] = np.cos(pos * div)

    Mts = np.eye(D, dtype=np.float32) + g["trend_w"].sum(0) + g["seas_w"].sum(0)
    bts = g["trend_b"].sum(0) + g["seas_b"].sum(0)

    WmM = Wm @ Mts                       # [80, 256]
    c0 = (bm[None] + pe) @ Mts + bts     # [10, 256]

    wqkv, bqkv = g["attn_wqkv"], g["attn_bqkv"]
    wo, bo = g["attn_wo"], g["attn_bo"]
    Wq1, Wk1, Wv1 = wqkv[0][:D], wqkv[0][D:2 * D], wqkv[0][2 * D:]
    bq1, bk1, bv1 = bqkv[0][:D], bqkv[0][D:2 * D], bqkv[0][2 * D:]
    Wq2, Wk2, Wv2 = wqkv[1][:D], wqkv[1][D:2 * D], wqkv[1][2 * D:]
    bq2, bk2, bv2 = bqkv[1][:D], bqkv[1][D:2 * D], bqkv[1][2 * D:]
    Wo1, bo1, Wo2, bo2 = wo[0], bo[0], wo[1], bo[1]

    def fm_chunks(w):          # [256, X] -> [128, 2, X] (K/feature chunks)
        return np.ascontiguousarray(
            w.reshape(2, 128, w.shape[1]).transpose(1, 0, 2))

    blk = (np.arange(NT) // BS)                             # [40] batch of token
    e1k = MBLK * (blk[None, :] == np.arange(BL)[:, None])   # [4, 40]
    e2q2 = np.tile(MBLK * np.eye(BL, dtype=np.float32), (1, 2))  # [4, 8]

    cb1 = np.zeros(CB1_SHAPE, np.float32)
    for rg in range(4):
        cb1[rg * 32:rg * 32 + BL, 0:40] = e1k
        cb1[rg * 32:rg * 32 + BL, 40:80] = e1k
        cb1[rg * 32:rg * 32 + BL, 80:88] = e2q2

    G = g["ln_g"][:, None] * g["fc_w"]
    cb2 = np.zeros(CB2_SHAPE, np.float32)
    cb2[:, 0:512] = fm_chunks(Wo1.T @ Wq2.T).reshape(128, 512)
    cb2[:, 512:1024] = fm_chunks(Wo1.T @ Wk2.T).reshape(128, 512)
    cb2[:, 1024:1536] = fm_chunks(Wo1.T @ Wv2.T).reshape(128, 512)
    cb2[:, 1536:2048] = fm_chunks(Wo2.T).reshape(128, 512)
    cb2[:, 2048:2056] = fm_chunks(Wo2.T @ G).reshape(128, 8)
    cb2[0, 2056:2312] = bo1 @ Wv2.T + bv2
    cb2[0, 2312:2568] = bo2
    cb2[0, 2568:2572] = bo2 @ G
    qbias = (bo1 @ Wq2.T + bq2).reshape(2, 128)
    kbias = (bo1 @ Wk2.T + bk2).reshape(2, 128)
    cb2[0:4, 2572:2700] = np.stack([qbias[0], qbias[1], kbias[0], kbias[1]])
    cb2[0, 2700:2704] = 1.0
    cb2[1, 2704:2708] = 1.0
    cb2[2, 2708:2748] = 1.0
    cb2[3, 2748:2788] = 1.0
    cb2[0:BL, 2788:2792] = np.tile(g["ln_g"] @ g["fc_w"], (BL, 1))
    cb2[0:BL, 2792:2796] = np.tile(g["ln_b"] @ g["fc_w"] + g["fc_b"], (BL, 1))

    com_q1 = WmM @ Wq1.T
    com_k1 = WmM @ Wk1.T
    com_v1 = WmM @ Wv1.T
    # folded L1 biases as 10 extra rows: row r applies to tokens t%10 == r
    bias_q1 = c0 @ Wq1.T + bq1           # [10, 256]
    bias_k1 = c0 @ Wk1.T + bk1
    bias_v1 = c0 @ Wv1.T + bv1
    onehot = np.tile(np.eye(BS, dtype=np.float32), (1, BL))  # [10, 40]

    # im2col of the live tokens: windows x[1988+t : 1993+t], t=0..9
    xp = np.zeros((B, S + 2, F), np.float32)
    xp[:, :S] = x
    xcol = np.stack([xp[:, S - BS - 2 + tt:S - BS + 3 + tt, :].reshape(B, 80)
                     for tt in range(BS)], 1)            # [B, 10, 80]
    xas = []
    for c in range(NCORES):
        xa = np.zeros(XA_SHAPE, np.float32)
        xa[0:80, 0:40] = xcol[c * BL:(c + 1) * BL].transpose(2, 0, 1).reshape(80, NT)
        xa[80:KI, 0:40] = onehot
        xa[0:80, 40:296] = com_q1
        xa[80:KI, 40:296] = bias_q1
        xa[0:80, 296:552] = com_k1
        xa[80:KI, 296:552] = bias_k1
        xa[0:80, 552:808] = com_v1
        xa[80:KI, 552:808] = bias_v1
        xas.append(xa.astype(np.float16))
    return cb1.astype(np.float16), cb2.astype(np.float16), xas


def kernel(**inputs):
    cb1, cb2, xas = _host_fold(inputs)
    if "nc" not in _MODULE_CACHE:
        nc = _build_module()
        if not nc.is_finalized():
            nc.finalize()
        _MODULE_CACHE["nc"] = nc
    nc = _MODULE_CACHE["nc"]
    in_maps = [{"cb1": cb1, "cb2": cb2, "xa": xas[c]} for c in range(NCORES)]
    trace = bool(os.environ.get("KERNEL_TRACE"))
    res = run_bass_kernel_spmd(nc, in_maps, core_ids=list(range(NCORES)),
                               trace=trace)
    LAST_RUN["result"] = res
    out = np.concatenate([res.results[c]["out"] for c in range(NCORES)], 0)
    return out.astype(np.float32)


# revision 10
# speedup vs baseline: 1.2461x; 1.0391x over previous
"""Trainium2 Bass kernel for nn_AdvancedAutoInformerModel.

Math (same folding as before): only the last block (10 tokens) of each batch
feeds the output, so all linear stages fold into small matrices on the host.
Per-core work (8 cores, 4 batches each): 40 token-columns, D=256.

v2 structure (vs the 31.5us baseline):
  * Everything the PE touches is fp16/bf16: fp16 streams 1 col/cycle vs 4
    for f32r on <256-col matmuls, LDWEIGHTS bytes halve, and the input DMA
    drops from 2.08MB to ~0.87MB per core.
  * DMA priority: xa (im2col + L1 weights) rides the sync ring alone while
    cb1 (bias-matmul e-block, 23KB) leads the gpsimd ring ahead of cb2
    (L2/head weights, 683KB) -- L1 compute starts ~6us earlier and cb2
    streams in during L1.
  * L1 q/k/v biases are folded into the projection matmuls as 10 extra
    im2col rows (onehot(t%10) x bias-table) -- no bias adds, no bias DMA.
  * The e-block holds only the 88 live columns (e1k|e1k, e2q2) instead of
    the 392-column tiling.
  * Sqrt's activation table is preloaded via a dummy activation during the
    DMA window so the LayerNorm tail never waits 1.3us for ACT_TABLE_LOAD.
  * PSUM->SBUF moves and the o=av*rb muls are split across gpsimd/vector;
    exp and sqrt live on scalar.
  * Head biases (bo2t|bgt) enter PSUM via a rank-1 ones matmul that runs as
    the accumulation-group opener, killing two tail STTs.
  * Block masking, 4-bank score tiles, feature-major denominators, the
    fused-STT LayerNorm tail, and the PE warmup are as before.
"""

import math
import os
import sys

import numpy as np
import ml_dtypes

for _p in ("/opt/trn_rl_repo",):
    if _p not in sys.path:
        sys.path.insert(0, _p)

import concourse.bass as bass
import concourse.bacc as bacc
import concourse.tile as tile
from concourse import mybir
from concourse.bass_utils import run_bass_kernel_spmd

F32 = mybir.dt.float32
FP16 = mybir.dt.float16
BF16 = mybir.dt.bfloat16

B, S, F, D, O, H, BS = 32, 2000, 16, 256, 4, 8, 10
HD = D // H                     # 32
NCORES = 8
BL = B // NCORES                # 4 batches per core
NT = BL * BS                    # 40 token-columns per core
KI = 80 + BS                    # im2col rows + 10 bias-onehot rows
SCALE = 1.0 / math.sqrt(HD)
ABIAS = 20.0 / SCALE            # block bias in raw-score units
MBLK = math.sqrt(ABIAS)         # one-hot scale so (M e)^T (M e) = ABIAS
CEXP = 16.5                     # global shift inside the exp

# xa  per-core fp16 [90, 808] = xcolT(40) | aq1(256) | ak1(256) | av1(256)
#     rows 80:90 carry onehot(t%10) (xcolT) / folded bias tables (aq1..av1)
# cb1 common fp16 [128, 88] = e1k(40) | e1k(40) | e2q2(8), replicated at
#     partitions rg*32 (rg=0..3) so each bank's bias matmul reads its rows
# cb2 common fp16 [128, 2796] = bq2(512) | bk2(512) | bv2(512) | wo2t(512) |
#     w2g(8) | cv2row(256) | hb(260) | qkbias4(128) | qkmask4(88) |
#     gfct(4) | cft(4)   (1/4-row blocks live at partition base 0)
XA_SHAPE = (KI, 808)
CB1_SHAPE = (128, 88)
CB2_SHAPE = (128, 2796)

_MODULE_CACHE = {}
LAST_RUN = {}
STAGE = 99       # truncate pipeline for HW bisection (debug only)
WARMUP = 6       # junk matmuls to open the PE clock gate
WCOLS = 256      # columns per junk matmul


def _build_module():
    nc = bacc.Bacc()
    xa_p = nc.declare_dram_parameter("xa", list(XA_SHAPE), FP16, isOutput=False)
    cb1_p = nc.declare_dram_parameter("cb1", list(CB1_SHAPE), FP16, isOutput=False)
    cb2_p = nc.declare_dram_parameter("cb2", list(CB2_SHAPE), FP16, isOutput=False)
    out_p = nc.declare_dram_parameter("out", [BL, O], F32, isOutput=True)

    with tile.TileContext(nc) as tc:
        with (
            tc.tile_pool(name="w", bufs=1) as wp,
            tc.tile_pool(name="act", bufs=1) as ap,
            tc.tile_pool(name="ps", bufs=1, space="PSUM") as pp,
        ):
            # ---- parameter DMAs: sync ring carries xa alone; gpsimd ring
            # carries the tiny e-block first, then the L2/head blob ----
            xa = wp.tile(list(XA_SHAPE), FP16, tag="xa", name="xa")
            cb1 = wp.tile(list(CB1_SHAPE), FP16, tag="cb1", name="cb1")
            cb2 = wp.tile(list(CB2_SHAPE), FP16, tag="cb2", name="cb2")
            nc.sync.dma_start(out=xa[0:45, :], in_=xa_p[0:45, :])
            nc.scalar.dma_start(out=xa[45:KI, :], in_=xa_p[45:KI, :])
            nc.gpsimd.dma_start(out=cb1, in_=cb1_p[:])
            nc.gpsimd.dma_start(out=cb2, in_=cb2_p[:])

            t = {
                "xcolT": xa[:, 0:40],
                "aq1": xa[:, 40:296], "ak1": xa[:, 296:552],
                "av1": xa[:, 552:808],
                "e1k": lambda rg: cb1[rg * 32:rg * 32 + BL, 0:40],
                "e1q": lambda rg: cb1[rg * 32:rg * 32 + BL, 0:80],
                "e2q": lambda rg: cb1[rg * 32:rg * 32 + BL, 80:88],
                "bq2": cb2[:, 0:512].rearrange("p (c x) -> p c x", c=2),
                "bk2": cb2[:, 512:1024].rearrange("p (c x) -> p c x", c=2),
                "bv2": cb2[:, 1024:1536].rearrange("p (c x) -> p c x", c=2),
                "wo2t": cb2[:, 1536:2048].rearrange("p (c x) -> p c x", c=2),
                "w2g": cb2[:, 2048:2056].rearrange("p (c x) -> p c x", c=2),
                "cv2row": cb2[0:1, 2056:2312],
                "hb": cb2[0:1, 2312:2572],
                "qkbias4": cb2[0:4, 2572:2700],
                "qkmask4": cb2[0:4, 2700:2788],
                "gfct": cb2[0:BL, 2788:2792],
                "cft": cb2[0:BL, 2792:2796],
            }

            def _early(probe):
                nc.gpsimd.dma_start(out=out_p[:], in_=probe)
            if STAGE <= 1:
                _early(cb1[0:BL, 0:O]); return nc

            # junk [40, 512] bf16 = warmup rhs; ones views serve the
            # denominator matmuls and the head's rank-1 bias matmul
            junk = ap.tile([NT, 512], BF16, tag="junk")
            nc.vector.memset(junk, 1.0)
            onesc = junk[:, 0:HD]

            # activation bias tiles (const pool only carries 0/1)
            negc = ap.tile([NT, 1], F32, tag="negc")
            nc.gpsimd.memset(negc, -CEXP)
            eps = ap.tile([BL, 1], F32, tag="eps")
            nc.gpsimd.memset(eps, 1e-5)

            # Preload the Exp activation table during the DMA window (the
            # scalar table slot holds one table; Sqrt's load then swaps in
            # under the LayerNorm tail's vector chain).
            scr = ap.tile([1, 1], F32, tag="scr")
            nc.scalar.activation(scr, junk[0:1, 0:1],
                                 mybir.ActivationFunctionType.Exp)

            # ================= layer 1 =================
            # Scores live in FOUR psum banks, one per PE row-group: bank rg
            # holds heads (rg, rg+4) at cols (0:nq, nq:2nq).
            st1 = pp.tile([NT, 4, 512], F32, tag="st", name="st1",
                          padded_shape=[NT, 4, 512])
            qk1ps = pp.tile([128, 160], F32, tag="qkps", name="qk1ps",
                            padded_shape=[128, 512])
            v1ps = pp.tile([NT, D], F32, tag="vps", name="v1ps",
                           padded_shape=[NT, 512])

            # PE warmup while the DMAs land so the HAM clock gate opens.
            # They write the banks the real layer-1 matmuls use, so
            # same-tile WAW ordering pins them FIRST on the PE.
            for wi in range(WARMUP - 2):
                nc.tensor.matmul(st1[0:HD, wi % 4, 0:WCOLS], onesc,
                                 junk[:, 0:WCOLS], start=True, stop=True,
                                 skip_group_check=True)
            nc.tensor.matmul(qk1ps[0:HD, :], onesc, junk[:, 0:160],
                             start=True, stop=True, skip_group_check=True)
            nc.tensor.matmul(v1ps[0:HD, :], onesc, junk[:, 0:256],
                             start=True, stop=True, skip_group_check=True)

            for rg in range(4):
                nc.tensor.matmul(st1[:, rg, 0:2 * NT], t["e1k"](rg),
                                 t["e1q"](rg), start=True,
                                 stop=False, skip_group_check=True,
                                 tile_position=(rg * 32, 0))

            # q1 | k1 feature-major into one PSUM tile [128, 160]; biases
            # ride the 10 extra im2col rows
            for mc in range(2):
                nc.tensor.matmul(qk1ps[:, mc * 40:(mc + 1) * 40],
                                 t["aq1"][:, mc * 128:(mc + 1) * 128],
                                 t["xcolT"], start=True, stop=True)
            for mc in range(2):
                nc.tensor.matmul(qk1ps[:, 80 + mc * 40:120 + mc * 40],
                                 t["ak1"][:, mc * 128:(mc + 1) * 128],
                                 t["xcolT"], start=True, stop=True)
            # v1 token-major [40, 256]
            nc.tensor.matmul(v1ps, t["xcolT"], t["av1"], start=True, stop=True)

            qk1 = ap.tile([128, 160], FP16, tag="qk1")
            nc.scalar.copy(out=qk1, in_=qk1ps)
            v1 = ap.tile([NT, D], FP16, tag="v1")
            nc.vector.tensor_copy(out=v1, in_=v1ps)

            if STAGE <= 2:
                _early(qk1[0:BL, 0:O]); return nc

            def attention(qk, v, st, nq, tag):
                """qk: [128, 2*nq | 2*NT] feature-major (q cols first),
                v: [NT, 256] fp16 token-major, st: [NT, 4, 512] scores psum
                (bank rg = heads rg, rg+4; block-bias already accumulated).
                Returns (o_chunk0, o_chunk1) [128, nq] fp16 feature-major."""
                # 8 QK matmuls accumulate into st (row-tiled);
                # head h -> bank h%4, col (h//4)*nq
                for h in range(H):
                    c, pb = h // 4, (h % 4) * HD
                    nc.tensor.matmul(
                        st[:, h % 4, c * nq:(c + 1) * nq],
                        qk[pb:pb + HD, 2 * nq + c * NT:2 * nq + (c + 1) * NT],
                        qk[pb:pb + HD, c * nq:(c + 1) * nq],
                        start=False, stop=True,
                        tile_position=(pb, 0), skip_group_check=True,
                    )
                # single Exp for all heads: exp(SCALE*s - C), bf16 out
                et = ap.tile([NT, 4, 2 * nq], BF16, tag=tag + "_et")
                nc.scalar.activation(et, st[:, :, 0:2 * nq],
                                     mybir.ActivationFunctionType.Exp,
                                     scale=SCALE, bias=negc)
                # denominators + weighted V, feature-major; head h = c*4+hh
                # lives in et[:, hh, c*nq:(c+1)*nq].  cs/av are single-bank
                # tiles shared by both chunks (col-groups repeat -> serial).
                cs = pp.tile([128, 2, nq], F32, tag="cs", name=tag + "cs",
                             padded_shape=[128, 2, 256])
                av = pp.tile([128, 2, nq], F32, tag="av", name=tag + "av",
                             padded_shape=[128, 2, 256])
                for c in range(2):
                    for hh in range(4):
                        nc.tensor.matmul(
                            cs[hh * HD:(hh + 1) * HD, c, :], onesc,
                            et[:, hh, c * nq:(c + 1) * nq],
                            start=True, stop=True, tile_position=(0, hh * HD),
                        )
                    for hh in range(4):
                        h = c * 4 + hh
                        nc.tensor.matmul(
                            av[hh * HD:(hh + 1) * HD, c, :],
                            v[0:NT, h * HD:(h + 1) * HD],
                            et[:, hh, c * nq:(c + 1) * nq],
                            start=True, stop=True, tile_position=(0, hh * HD),
                        )
                # one reciprocal + one mul cover both chunks (DVE time is
                # overhead-dominated at this size)
                rb = ap.tile([128, 2, nq], F32, tag=tag + "_rb")
                nc.vector.reciprocal_approx_fast(out=rb, in_=cs[:, :, :])
                ob = ap.tile([128, 2, nq], FP16, tag=tag + "_o")
                nc.vector.tensor_mul(ob, av[:, :, :], rb)
                return [ob[:, 0, :], ob[:, 1, :]]

            o1 = attention(qk1, v1, st1, NT, "l1")
            if STAGE <= 3:
                _early(o1[0][0:BL, 0:O]); return nc

            # ================= layer 2 =================
            # q2 (last token of each batch) + k2 share one PSUM tile.
            # NB: start=True resets has_written for the WHOLE psum bank, so
            # each col-region's accumulation group must complete before the
            # next region's start (mc outer, kc inner).
            qk2ps = pp.tile([128, 88], F32, tag="qkps", name="qk2ps",
                            padded_shape=[128, 512])
            v2ps = pp.tile([NT, D], F32, tag="vps", name="v2ps",
                           padded_shape=[NT, 512])
            ols = [o1[kc].rearrange("p (b t) -> p b t", t=BS)[:, :, BS - 1]
                   for kc in range(2)]
            # rank-4 bias opener: qkbias4.T @ qkmask4 paints the whole
            # [128, 88] bias table into PSUM; projections then accumulate
            nc.tensor.matmul(qk2ps[:, 0:88], t["qkbias4"], t["qkmask4"],
                             start=True, stop=False, skip_group_check=True)
            for mc in range(2):
                for kc in range(2):
                    nc.tensor.matmul(qk2ps[:, mc * 4:(mc + 1) * 4],
                                     t["bq2"][:, kc, mc * 128:(mc + 1) * 128],
                                     ols[kc], start=False, stop=False,
                                     skip_group_check=True)
            for mc in range(2):
                for kc in range(2):
                    nc.tensor.matmul(qk2ps[:, 8 + mc * 40:48 + mc * 40],
                                     t["bk2"][:, kc, mc * 128:(mc + 1) * 128],
                                     o1[kc], start=False,
                                     stop=(mc == 1 and kc == 1),
                                     skip_group_check=True)
            nc.tensor.matmul(v2ps, junk[0:1, 0:NT], t["cv2row"],
                             start=True, stop=False, skip_group_check=True)
            for kc in range(2):
                nc.tensor.matmul(v2ps, o1[kc], t["bv2"][:, kc, :],
                                 start=False, stop=(kc == 1),
                                 skip_group_check=True)

            qk2 = ap.tile([128, 88], FP16, tag="qk2")
            nc.scalar.copy(out=qk2, in_=qk2ps)
            v2 = ap.tile([NT, D], FP16, tag="v2")
            nc.vector.tensor_copy(out=v2, in_=v2ps)

            st2 = pp.tile([NT, 4, 512], F32, tag="st", name="st2",
                          padded_shape=[NT, 4, 512])
            for rg in range(4):
                nc.tensor.matmul(st2[:, rg, 0:2 * BL], t["e1k"](rg),
                                 t["e2q"](rg), start=True,
                                 stop=False, skip_group_check=True,
                                 tile_position=(rg * 32, 0))
            o2 = attention(qk2, v2, st2, BL, "l2")
            if STAGE <= 5:
                _early(o2[0][0:BL, 0:O]); return nc

            # ================= head =================
            # h2 [4, 256] token-major + t2 [4, 4] in one PSUM tile; the
            # rank-1 ones x (bo2t|bgt) matmul opens the accumulation group
            # so biases are already inside PSUM.
            hd = pp.tile([BL, D + O], F32, tag="qkps", name="headps",
                         padded_shape=[BL, 512])
            nc.tensor.matmul(hd[:, 0:D + O], junk[0:1, 0:BL], t["hb"],
                             start=True, stop=False, skip_group_check=True)
            for kc in range(2):
                nc.tensor.matmul(hd[:, 0:D], o2[kc], t["wo2t"][:, kc, :],
                                 start=False, stop=False,
                                 skip_group_check=True)
            for kc in range(2):
                nc.tensor.matmul(hd[:, D:D + O], o2[kc], t["w2g"][:, kc, :],
                                 start=False, stop=(kc == 1),
                                 skip_group_check=True)

            M = mybir.AluOpType.mult
            SUB = mybir.AluOpType.subtract
            h2 = ap.tile([BL, D], F32, tag="h2")
            su = ap.tile([BL, 1], F32, tag="su")
            nc.vector.tensor_scalar(out=h2, in0=hd[:, 0:D], scalar1=1.0,
                                    scalar2=0.0, op0=M,
                                    op1=mybir.AluOpType.add, accum_out=su)
            sq = ap.tile([BL, D], F32, tag="sq")
            sqs = ap.tile([BL, 1], F32, tag="sqs")
            nc.vector.scalar_tensor_tensor(out=sq, in0=h2, scalar=1.0, in1=h2,
                                           op0=M, op1=M, accum_out=sqs)
            # var = sqs/256 - (su/256)^2 ; rstd = 1/sqrt(var + eps)
            musq = ap.tile([BL, 1], F32, tag="musq")
            nc.vector.tensor_scalar(out=musq, in0=su, scalar1=su,
                                    scalar2=1.0 / (D * D), op0=M, op1=M)
            var = ap.tile([BL, 1], F32, tag="var")
            nc.vector.scalar_tensor_tensor(out=var, in0=sqs, scalar=1.0 / D,
                                           in1=musq, op0=M, op1=SUB)
            std = ap.tile([BL, 1], F32, tag="std")
            nc.scalar.activation(std, var, mybir.ActivationFunctionType.Sqrt,
                                 bias=eps)
            rstd = ap.tile([BL, 1], F32, tag="rstd")
            nc.vector.reciprocal_approx_fast(out=rstd, in_=std)
            u = ap.tile([BL, 1], F32, tag="u")
            nc.vector.tensor_scalar(out=u, in0=rstd, scalar1=su,
                                    scalar2=1.0 / D, op0=M, op1=M)
            # r2 = rstd*t2' - (u*gfct - cft)   (t2' = o2@W2g + bgt)
            bm = ap.tile([BL, O], F32, tag="bm")
            nc.vector.scalar_tensor_tensor(out=bm, in0=t["gfct"], scalar=u,
                                           in1=t["cft"], op0=M, op1=SUB)
            r2 = ap.tile([BL, O], F32, tag="r2")
            nc.vector.scalar_tensor_tensor(out=r2, in0=hd[:, D:D + O],
                                           scalar=rstd, in1=bm,
                                           op0=M, op1=SUB)
            nc.sync.dma_start(out=out_p[:], in_=r2)

    return nc


def _host_fold(inputs):
    """Fold all linear stages; returns (cb1, cb2, xa per-core list), fp16."""
    g = {k: np.asarray(v, np.float32) for k, v in inputs.items()}
    x = g["x"]

    Wm = np.zeros((5, F, D), np.float32)
    w1, w3, w5 = g["conv_w1"], g["conv_w3"], g["conv_w5"]
    Wm[0] = w5[0]
    Wm[1] = w3[0] + w5[1]
    Wm[2] = w1[0] + w3[1] + w5[2]
    Wm[3] = w3[2] + w5[3]
    Wm[4] = w5[4]
    Wm = Wm.reshape(80, D)
    bm = g["conv_b1"] + g["conv_b3"] + g["conv_b5"]

    toks = np.arange(S - BS, S)
    pos = toks.astype(np.float32)[:, None]
    div = np.exp(np.arange(0, D, 2, dtype=np.float32) * (-math.log(10000.0) / D))
    pe = np.zeros((BS, D), np.float32)
    pe[:, 0::2] = np.sin(pos * div)
    pe[:, 1::2] = np.cos(pos * div)

    Mts = np.eye(D, dtype=np.float32) + g["trend_w"].sum(0) + g["seas_w"].sum(0)
    bts = g["trend_b"].sum(0) + g["seas_b"].sum(0)

    WmM = Wm @ Mts                       # [80, 256]
    c0 = (bm[None] + pe) @ Mts + bts     # [10, 256]

    wqkv, bqkv = g["attn_wqkv"], g["attn_bqkv"]
    wo, bo = g["attn_wo"], g["attn_bo"]
    Wq1, Wk1, Wv1 = wqkv[0][:D], wqkv[0][D:2 * D], wqkv[0][2 * D:]
    bq1, bk1, bv1 = bqkv[0][:D], bqkv[0][D:2 * D], bqkv[0][2 * D:]
    Wq2, Wk2, Wv2 = wqkv[1][:D], wqkv[1][D:2 * D], wqkv[1][2 * D:]
    bq2, bk2, bv2 = bqkv[1][:D], bqkv[1][D:2 * D], bqkv[1][2 * D:]
    Wo1, bo1, Wo2, bo2 = wo[0], bo[0], wo[1], bo[1]

    def fm_chunks(w):          # [256, X] -> [128, 2, X] (K/feature chunks)
        return np.ascontiguousarray(
            w.reshape(2, 128, w.shape[1]).transpose(1, 0, 2))

    blk = (np.arange(NT) // BS)                             # [40] batch of token
    e1k = MBLK * (blk[None, :] == np.arange(BL)[:, None])   # [4, 40]
    e2q2 = np.tile(MBLK * np.eye(BL, dtype=np.float32), (1, 2))  # [4, 8]

    cb1 = np.zeros(CB1_SHAPE, np.float32)
    for rg in range(4):
        cb1[rg * 32:rg * 32 + BL, 0:40] = e1k
        cb1[rg * 32:rg * 32 + BL, 40:80] = e1k
        cb1[rg * 32:rg * 32 + BL, 80:88] = e2q2

    G = g["ln_g"][:, None] * g["fc_w"]
    cb2 = np.zeros(CB2_SHAPE, np.float32)
    cb2[:, 0:512] = fm_chunks(Wo1.T @ Wq2.T).reshape(128, 512)
    cb2[:, 512:1024] = fm_chunks(Wo1.T @ Wk2.T).reshape(128, 512)
    cb2[:, 1024:1536] = fm_chunks(Wo1.T @ Wv2.T).reshape(128, 512)
    cb2[:, 1536:2048] = fm_chunks(Wo2.T).reshape(128, 512)
    cb2[:, 2048:2056] = fm_chunks(Wo2.T @ G).reshape(128, 8)
    cb2[0, 2056:2312] = bo1 @ Wv2.T + bv2
    cb2[0, 2312:2568] = bo2
    cb2[0, 2568:2572] = bo2 @ G
    qbias = (bo1 @ Wq2.T + bq2).reshape(2, 128)
    kbias = (bo1 @ Wk2.T + bk2).reshape(2, 128)
    cb2[0:4, 2572:2700] = np.stack([qbias[0], qbias[1], kbias[0], kbias[1]])
    cb2[0, 2700:2704] = 1.0
    cb2[1, 2704:2708] = 1.0
    cb2[2, 2708:2748] = 1.0
    cb2[3, 2748:2788] = 1.0
    cb2[0:BL, 2788:2792] = np.tile(g["ln_g"] @ g["fc_w"], (BL, 1))
    cb2[0:BL, 2792:2796] = np.tile(g["ln_b"] @ g["fc_w"] + g["fc_b"], (BL, 1))

    com_q1 = WmM @ Wq1.T
    com_k1 = WmM @ Wk1.T
    com_v1 = WmM @ Wv1.T
    # folded L1 biases as 10 extra rows: row r applies to tokens t%10 == r
    bias_q1 = c0 @ Wq1.T + bq1           # [10, 256]
    bias_k1 = c0 @ Wk1.T + bk1
    bias_v1 = c0 @ Wv1.T + bv1
    onehot = np.tile(np.eye(BS, dtype=np.float32), (1, BL))  # [10, 40]

    # im2col of the live tokens: windows x[1988+t : 1993+t], t=0..9
    xp = np.zeros((B, S + 2, F), np.float32)
    xp[:, :S] = x
    xcol = np.stack([xp[:, S - BS - 2 + tt:S - BS + 3 + tt, :].reshape(B, 80)
                     for tt in range(BS)], 1)            # [B, 10, 80]
    xas = []
    for c in range(NCORES):
        xa = np.zeros(XA_SHAPE, np.float32)
        xa[0:80, 0:40] = xcol[c * BL:(c + 1) * BL].transpose(2, 0, 1).reshape(80, NT)
        xa[80:KI, 0:40] = onehot
        xa[0:80, 40:296] = com_q1
        xa[80:KI, 40:296] = bias_q1
        xa[0:80, 296:552] = com_k1
        xa[80:KI, 296:552] = bias_k1
        xa[0:80, 552:808] = com_v1
        xa[80:KI, 552:808] = bias_v1
        xas.append(xa.astype(np.float16))
    return cb1.astype(np.float16), cb2.astype(np.float16), xas


def kernel(**inputs):
    cb1, cb2, xas = _host_fold(inputs)
    if "nc" not in _MODULE_CACHE:
        nc = _build_module()
        if not nc.is_finalized():
            nc.finalize()
        _MODULE_CACHE["nc"] = nc
    nc = _MODULE_CACHE["nc"]
    in_maps = [{"cb1": cb1, "cb2": cb2, "xa": xas[c]} for c in range(NCORES)]
    trace = bool(os.environ.get("KERNEL_TRACE"))
    res = run_bass_kernel_spmd(nc, in_maps, core_ids=list(range(NCORES)),
                               trace=trace)
    LAST_RUN["result"] = res
    out = np.concatenate([res.results[c]["out"] for c in range(NCORES)], 0)
    return out.astype(np.float32)
